# revision 12
# baseline (speedup 1.0000x reference)
"""Trainium2 Bass kernel for ActiveMatter NPINN PDE loss.

Computes (total, loss_cont, loss_conc, loss_dxx) for the upwind-convection /
diffusion / continuity PINN residuals over inputs u, v, c, Dxx of shape
(4, 22, 256, 256) fp32.

Sharding: 8 cores, core i <- (batch b = i//2, time-half h = i%2).  Each core
handles 10 interior time frames; c/Dxx need a +-1 frame halo (12 frames).
Each core reduces its residual squares to 3 partial sums; the host adds the
8 partial triples and forms the final scalars.

Layout: each 256x256 frame is packed [128 part, 2 blk, 256 w] (+-2 col halo
-> 260 per blk) with rows h = 128*blk + p.  W-shifts are free-dim offset
reads (odd shifts via a one-column-shifted bf16 copy to keep 4B alignment
for the DVE 2x mode); H-shifts/stencils are PE matmuls with circulant
block matrices (main 128x128 band + cross-block wrap correction).

All math 4*DX-scaled; see _build for the fused op list.
"""

import sys

for _p in ("/opt/trn_rl_repo",):
    if _p not in sys.path:
        sys.path.insert(0, _p)

import numpy as np

import concourse.bass as bass
import concourse.bacc as bacc
import concourse.mybir as mybir
from concourse.tile import TileContext
from concourse.bass_utils import run_bass_kernel_spmd

# ---------------------------------------------------------------- constants
B, T, H, W = 4, 22, 256, 256
N_CORES = 8
T_INT = 10           # interior frames per core
T_HALO = T_INT + 2   # c/Dxx frames per core
DX = 10.0 / 256.0
DT = 0.25
D_T = 0.05
BETA = D_T / DX                     # 1.28
KT = 8.0 * DX                       # df_dt coeff on (f_next - f_prev), 4x scale
F32 = mybir.dt.float32
BF = mybir.dt.bfloat16
AL = mybir.AluOpType
AF = mybir.ActivationFunctionType

_CACHE = {}


# ------------------------------------------------------- stencil matrices
def _circ_lhst(taps: dict) -> np.ndarray:
    """[2, 128, 128] lhsT blocks (main, corr) for the periodic row stencil
    out[h] = sum_s taps[s] * f[(h+s) % 256] on 256 rows packed as two
    128-row blocks."""
    M = np.zeros((256, 256), np.float64)
    for s, a in taps.items():
        for h in range(256):
            M[h, (h + s) % 256] += a
    A = M[:128, :128]
    C = M[:128, 128:256]
    assert np.allclose(M[128:, 128:], A) and np.allclose(M[128:, :128], C)
    return np.stack([A.T, C.T]).astype(np.float32)


def _stencil_mats() -> np.ndarray:
    mats = [
        _circ_lhst({1: 2.0, -1: -2.0}),                          # 0 dvy (x2)
        _circ_lhst({0: 1.0, 1: 1.0}),                            # 1 vcn
        _circ_lhst({0: 3.0, -1: -1.0}),                          # 2 gp
        _circ_lhst({1: 3.0, 2: -1.0}),                           # 3 gm
        _circ_lhst({1: 4 * BETA, 0: -8 * BETA, -1: 4 * BETA}),   # 4 lape
        _circ_lhst({-1: 1.0}),                                   # 5 sm
    ]
    # [6, 2, 128, 128] -> [128, 6, 2, 128]
    return np.ascontiguousarray(np.transpose(np.stack(mats), (2, 0, 1, 3)))


# ------------------------------------------------------------ graph build
def _build():
    nc = bacc.Bacc("TRN2")
    u_ext = nc.declare_dram_parameter("u", [T_INT, H, W], F32, isOutput=False)
    v_ext = nc.declare_dram_parameter("v", [T_INT, H, W], F32, isOutput=False)
    c_ext = nc.declare_dram_parameter("c", [T_HALO, H, W], F32, isOutput=False)
    d_ext = nc.declare_dram_parameter("d", [T_HALO, H, W], F32, isOutput=False)
    sm_ext = nc.declare_dram_parameter("stmat", [128, 6, 2, 128], F32, isOutput=False)
    out_ext = nc.declare_dram_parameter("out", [1, 4], F32, isOutput=True)

    with TileContext(nc) as tc:
        with (
            tc.tile_pool(name="const", bufs=1) as constp,
            tc.tile_pool(name="frames", bufs=1) as framesp,
            tc.tile_pool(name="odds", bufs=3) as oddp,
            tc.tile_pool(name="scr", bufs=14) as scr,
            tc.tile_pool(name="psum", bufs=1, space="PSUM") as psp,
        ):
            stmat = constp.tile([128, 6, 2, 128], BF, name="stmat_sb")
            nc.gpsimd.dma_start(out=stmat, in_=sm_ext[:, :, :, :])

            ones = constp.tile([128, 1], F32, name="ones_sb")
            nc.vector.memset(ones, 1.0)

            accs = constp.tile([128, 3, T_INT], F32, name="accs")

            def dram_frames(ext):
                return ext.rearrange("t (j p) w -> t p j w", p=128)

            du, dv = dram_frames(u_ext), dram_frames(v_ext)
            dc, dd = dram_frames(c_ext), dram_frames(d_ext)

            def load_ext(name, src, t):
                tile = framesp.tile([128, 2, 260], BF, name=f"{name}{t}")
                nc.gpsimd.dma_start(out=tile[:, :, 2:258], in_=src[t])
                nc.gpsimd.tensor_copy(tile[:, :, 0:2], tile[:, :, 256:258])
                nc.gpsimd.tensor_copy(tile[:, :, 258:260], tile[:, :, 2:4])
                return tile

            ut = [load_ext("u", du, t) for t in range(T_INT)]
            vt = []
            for t in range(T_INT):
                v_tile = framesp.tile([128, 2, 256], BF, name=f"v{t}")
                nc.gpsimd.dma_start(out=v_tile, in_=dv[t])
                vt.append(v_tile)
            ct = [load_ext("c", dc, t) for t in range(T_HALO)]
            dt_ = [load_ext("d", dd, t) for t in range(T_HALO)]

            def mkodd(src, nm):
                """One-col-right-shifted copy: odd[., c] = src[., c-1].
                On ScalarE (ACT) — always 1x there, frees VectorE."""
                o = oddp.tile([128, 2, 260], BF, name=nm, tag=nm[0] + "o")
                nc.scalar.copy(o[:, :, 1:260], src[:, :, 0:259])
                return o

            def rd(ext_tile, k):     # k even
                return ext_tile[:, :, 2 + k : 258 + k]

            def rdo(odd_tile, k):    # k odd
                return odd_tile[:, :, 3 + k : 259 + k]

            def w(name, dtype=BF):
                return scr.tile([128, 2, 260], dtype, name=name, tag="w")

            def stencil(ps, k, rhs3d, start, stop):
                nc.tensor.matmul(ps[:, :, :], stmat[:, k, 0, :], rhs3d,
                                 start=start, stop=False)
                nc.tensor.matmul(ps[:, 0, :], stmat[:, k, 1, :], rhs3d[:, 1, :],
                                 start=False, stop=False)
                nc.tensor.matmul(ps[:, 1, :], stmat[:, k, 1, :], rhs3d[:, 0, :],
                                 start=False, stop=stop)

            STT = nc.vector.scalar_tensor_tensor
            TT = nc.vector.tensor_tensor

            for t in range(T_INT):
                u, v = ut[t], vt[t]
                uo = mkodd(u, f"uo{t}")
                # ---------------- shared per-frame fields
                uc2 = w("uc2")
                uc2v = uc2[:, :, 0:256]
                TT(uc2v, rd(u, 0), rdo(uo, 1), AL.add)
                ucp = w("ucp")[:, :, 0:256]
                nc.vector.tensor_scalar_max(ucp, uc2v, 0.0)
                ucm = w("ucm")[:, :, 0:256]
                nc.vector.tensor_scalar_min(ucm, uc2v, 0.0)

                dvy = psp.tile([128, 2, 256], F32, name="dvy", tag="dvy")
                stencil(dvy, 0, v[:, :, :], True, True)

                dvx = w("dvx")[:, :, 0:256]
                nc.gpsimd.tensor_tensor(dvx, rdo(uo, 1), rdo(uo, -1), AL.subtract)
                dvs4 = w("dvs4")[:, :, 0:256]
                STT(dvs4, dvx, 2.0, dvy, AL.mult, AL.add)

                vcnp = psp.tile([128, 2, 256], F32, name="vcnp", tag="vcn")
                stencil(vcnp, 1, v[:, :, :], True, True)
                vcn2 = w("vcn2")[:, :, 0:256]
                nc.scalar.copy(vcn2, vcnp)

                sq0 = w("sq0")[:, :, 0:256]
                nc.scalar.activation(sq0, dvs4, AF.Square,
                                     accum_out=accs[:, 0, t : t + 1])

                # ---------------- per-field residual
                for fi, ftiles in ((1, ct), (2, dt_)):
                    f0, fprev, fnext = ftiles[t + 1], ftiles[t], ftiles[t + 2]
                    fo = mkodd(f0, f"fo{fi}{t}")
                    fp = w("fp")[:, :, 0:256]
                    STT(fp, rd(f0, 0), 3.0, rdo(fo, -1), AL.mult, AL.subtract)
                    fm = w("fm")[:, :, 0:256]
                    STT(fm, rdo(fo, 1), 3.0, rd(f0, 2), AL.mult, AL.subtract)
                    p = w("p")[:, :, 0:256]
                    TT(p, ucp, fp, AL.mult)
                    q = w("q")[:, :, 0:256]
                    TT(q, ucm, fm, AL.mult)
                    s1 = w("s1")[:, :, 0:256]
                    TT(s1, p, q, AL.add)
                    dd = w("dd")[:, :, 0:256]
                    TT(dd, rdo(fo, 1), rd(f0, 0), AL.subtract)
                    # phiN = 4*beta*D - (p+q) = -phi'
                    phi = w("phi")
                    STT(phi[:, :, 2:258], dd, 4.0 * BETA, s1, AL.mult, AL.subtract)
                    po = oddp.tile([128, 2, 260], BF, name=f"po{fi}{t}", tag="po")
                    nc.scalar.copy(po[:, :, 3:259], phi[:, :, 2:258])
                    nc.scalar.copy(po[:, :, 2:3], phi[:, :, 257:258])
                    # cdx = phi' - phi'_im1 = phiN_im1 - phiN
                    cdx = w("cdx")[:, :, 0:256]
                    TT(cdx, rdo(po, -1), phi[:, :, 2:258], AL.subtract)

                    gp = psp.tile([128, 2, 256], F32, name="gp", tag="gp", bufs=2)
                    stencil(gp, 2, rd(f0, 0), True, True)
                    gm = psp.tile([128, 2, 256], F32, name="gm", tag="gm", bufs=2)
                    stencil(gm, 3, rd(f0, 0), True, True)

                    p2 = w("p2")[:, :, 0:256]
                    STT(p2, vcn2, 0.0, gp, AL.max, AL.mult)
                    q2 = w("q2")[:, :, 0:256]
                    STT(q2, vcn2, 0.0, gm, AL.min, AL.mult)
                    psi = w("psi")[:, :, 0:256]
                    TT(psi, p2, q2, AL.add)

                    py = psp.tile([128, 2, 256], F32, name="py", tag="py", bufs=2)
                    stencil(py, 5, psi, True, False)
                    stencil(py, 4, rd(f0, 0), False, True)

                    t2 = w("t2")[:, :, 0:256]
                    TT(t2, psi, py, AL.subtract)
                    t3 = w("t3")[:, :, 0:256]
                    TT(t3, cdx, t2, AL.add)
                    pd = w("pd")[:, :, 0:256]
                    TT(pd, rd(f0, 0), dvs4, AL.mult)
                    ra = w("ra")[:, :, 0:256]
                    TT(ra, t3, pd, AL.subtract)
                    dtt = w("dtt")[:, :, 0:256]
                    nc.gpsimd.tensor_tensor(dtt, rd(fnext, 0), rd(fprev, 0),
                                            AL.subtract)
                    dttk = w("dttk")[:, :, 0:256]
                    nc.gpsimd.tensor_scalar_mul(dttk, dtt, KT)
                    rr = w("rr")[:, :, 0:256]
                    TT(rr, ra, dttk, AL.add)
                    sqf = w("sqf")[:, :, 0:256]
                    nc.scalar.activation(sqf, rr, AF.Square,
                                         accum_out=accs[:, fi, t : t + 1])

            # ---------------- final reduction to [1, 3]
            red3 = constp.tile([128, 3], F32, name="red3")
            for k in range(3):
                nc.vector.tensor_reduce(red3[:, k : k + 1], accs[:, k, :],
                                        mybir.AxisListType.X, AL.add)
            psr = psp.tile([1, 4], F32, name="psr", tag="dvy")
            nc.tensor.matmul(psr[:, 0:3], ones, red3, start=True, stop=True)
            outt = constp.tile([1, 4], F32, name="outt")
            nc.vector.memset(outt, 0.0)
            nc.scalar.copy(outt[:, 0:3], psr[:, 0:3])
            nc.sync.dma_start(out=out_ext[:, :], in_=outt)

    nc.compile()
    return nc


def _get_nc():
    if "nc" not in _CACHE:
        _CACHE["nc"] = _build()
        _CACHE["stmat"] = _stencil_mats()
    return _CACHE["nc"]


def _make_in_maps(u, v, c, Dxx):
    u = np.ascontiguousarray(np.asarray(u, dtype=np.float32))
    v = np.ascontiguousarray(np.asarray(v, dtype=np.float32))
    c = np.ascontiguousarray(np.asarray(c, dtype=np.float32))
    Dxx = np.ascontiguousarray(np.asarray(Dxx, dtype=np.float32))
    stmat = _CACHE["stmat"]
    in_maps = []
    for i in range(N_CORES):
        b, h = i // 2, i % 2
        t0 = 1 + T_INT * h
        in_maps.append({
            "u": np.ascontiguousarray(u[b, t0 : t0 + T_INT]),
            "v": np.ascontiguousarray(v[b, t0 : t0 + T_INT]),
            "c": np.ascontiguousarray(c[b, t0 - 1 : t0 + T_INT + 1]),
            "d": np.ascontiguousarray(Dxx[b, t0 - 1 : t0 + T_INT + 1]),
            "stmat": stmat,
        })
    return in_maps


def _combine(results):
    s = np.zeros(3, dtype=np.float64)
    for r in results:
        s += np.asarray(r["out"], dtype=np.float64)[0, :3]
    n = B * (T - 2) * H * W
    loss_cont = s[0] / (16.0 * DX * DX * n)
    loss_conc = s[1] / (16.0 * DX * DX * n)
    loss_dxx = s[2] / (16.0 * DX * DX * n)
    total = loss_cont + loss_conc + loss_dxx
    return np.array([total, loss_cont, loss_conc, loss_dxx], dtype=np.float32)


def kernel(u, v, c, Dxx):
    nc = _get_nc()
    in_maps = _make_in_maps(u, v, c, Dxx)
    res = run_bass_kernel_spmd(nc, in_maps, core_ids=list(range(N_CORES)))
    return _combine(res.results)


if __name__ == "__main__":
    rng = np.random.default_rng(0)
    inputs = {
        "u": rng.standard_normal((B, T, H, W), dtype=np.float32),
        "v": rng.standard_normal((B, T, H, W), dtype=np.float32),
        "c": rng.random((B, T, H, W), dtype=np.float32),
        "Dxx": rng.random((B, T, H, W), dtype=np.float32),
    }
    print(kernel(**inputs))


# revision 13
# speedup vs baseline: 1.3447x; 1.3447x over previous
"""Trainium2 Bass kernel for ActiveMatter NPINN PDE loss.

Computes (total, loss_cont, loss_conc, loss_dxx) for the upwind-convection /
diffusion / continuity PINN residuals over inputs u, v, c, Dxx of shape
(4, 22, 256, 256) fp32.

Sharding: 8 cores, core i <- (batch b = i//2, time-half h = i%2).  Each core
handles 10 interior time frames; c/Dxx need a +-1 frame halo (12 frames).
Each core reduces its residual squares to 3 partial sums; the host adds the
8 partial triples and forms the final scalars.

Layout: each 256x256 frame is packed [128 part, 2 blk, 256 w] (+-2 col halo
-> 260 per blk) with rows h = 128*blk + p.  W-shifts are free-dim offset
reads (odd shifts via a one-column-shifted bf16 copy to keep 4B alignment
for the DVE 2x mode); H-shifts/stencils are PE matmuls with circulant
block matrices (main 128x128 band + cross-block wrap correction).

All math 4*DX-scaled; see _build for the fused op list.
"""

import sys

for _p in ("/opt/trn_rl_repo",):
    if _p not in sys.path:
        sys.path.insert(0, _p)

import numpy as np

import concourse.bass as bass
import concourse.bacc as bacc
import concourse.mybir as mybir
from concourse.tile import TileContext
from concourse.bass_utils import run_bass_kernel_spmd

# ---------------------------------------------------------------- constants
B, T, H, W = 4, 22, 256, 256
N_CORES = 8
T_INT = 10           # interior frames per core
T_HALO = T_INT + 2   # c/Dxx frames per core
DX = 10.0 / 256.0
DT = 0.25
D_T = 0.05
BETA = D_T / DX                     # 1.28
KT = 8.0 * DX                       # df_dt coeff on (f_next - f_prev), 4x scale
F32 = mybir.dt.float32
BF = mybir.dt.bfloat16
AL = mybir.AluOpType
AF = mybir.ActivationFunctionType

_CACHE = {}


# ------------------------------------------------------- stencil matrices
def _circ_lhst(taps: dict) -> np.ndarray:
    """[2, 128, 128] lhsT blocks (main, corr) for the periodic row stencil
    out[h] = sum_s taps[s] * f[(h+s) % 256] on 256 rows packed as two
    128-row blocks."""
    M = np.zeros((256, 256), np.float64)
    for s, a in taps.items():
        for h in range(256):
            M[h, (h + s) % 256] += a
    A = M[:128, :128]
    C = M[:128, 128:256]
    assert np.allclose(M[128:, 128:], A) and np.allclose(M[128:, :128], C)
    return np.stack([A.T, C.T]).astype(np.float32)


def _stencil_mats() -> np.ndarray:
    mats = [
        _circ_lhst({1: 2.0, -1: -2.0}),                          # 0 dvy (x2)
        _circ_lhst({0: 1.0, 1: 1.0}),                            # 1 vcn
        _circ_lhst({0: 3.0, -1: -1.0}),                          # 2 gp
        _circ_lhst({1: 3.0, 2: -1.0}),                           # 3 gm
        _circ_lhst({1: 4 * BETA, 0: -8 * BETA, -1: 4 * BETA}),   # 4 lape
        _circ_lhst({-1: 1.0}),                                   # 5 sm
    ]
    # [6, 2, 128, 128] -> [128, 6, 2, 128]
    return np.ascontiguousarray(np.transpose(np.stack(mats), (2, 0, 1, 3)))


# ------------------------------------------------------------ graph build
def _build():
    nc = bacc.Bacc("TRN2")
    u_ext = nc.declare_dram_parameter("u", [T_INT, H, W], F32, isOutput=False)
    v_ext = nc.declare_dram_parameter("v", [T_INT, H, W], F32, isOutput=False)
    c_ext = nc.declare_dram_parameter("c", [T_HALO, H, W], F32, isOutput=False)
    d_ext = nc.declare_dram_parameter("d", [T_HALO, H, W], F32, isOutput=False)
    sm_ext = nc.declare_dram_parameter("stmat", [128, 6, 2, 128], F32, isOutput=False)
    out_ext = nc.declare_dram_parameter("out", [1, 4], F32, isOutput=True)

    with TileContext(nc) as tc:
        with (
            tc.tile_pool(name="const", bufs=1) as constp,
            tc.tile_pool(name="frames", bufs=1) as framesp,
            tc.tile_pool(name="odds", bufs=3) as oddp,
            tc.tile_pool(name="scr", bufs=14) as scr,
            tc.tile_pool(name="psum", bufs=1, space="PSUM") as psp,
        ):
            stmat = constp.tile([128, 6, 2, 128], BF, name="stmat_sb")
            nc.gpsimd.dma_start(out=stmat, in_=sm_ext[:, :, :, :])

            ones = constp.tile([128, 1], F32, name="ones_sb")
            nc.vector.memset(ones, 1.0)

            accs = constp.tile([128, 3, T_INT], F32, name="accs")

            def dram_frames(ext):
                return ext.rearrange("t (j p) w -> t p j w", p=128)

            du, dv = dram_frames(u_ext), dram_frames(v_ext)
            dc, dd = dram_frames(c_ext), dram_frames(d_ext)

            def load_ext(name, src, t):
                tile = framesp.tile([128, 2, 260], BF, name=f"{name}{t}")
                nc.gpsimd.dma_start(out=tile[:, :, 2:258], in_=src[t])
                nc.gpsimd.tensor_copy(tile[:, :, 0:2], tile[:, :, 256:258])
                nc.gpsimd.tensor_copy(tile[:, :, 258:260], tile[:, :, 2:4])
                return tile

            ut = [load_ext("u", du, t) for t in range(T_INT)]
            vt = []
            for t in range(T_INT):
                v_tile = framesp.tile([128, 2, 256], BF, name=f"v{t}")
                nc.gpsimd.dma_start(out=v_tile, in_=dv[t])
                vt.append(v_tile)
            ct = [load_ext("c", dc, t) for t in range(T_HALO)]
            dt_ = [load_ext("d", dd, t) for t in range(T_HALO)]

            def mkodd(src, nm):
                """One-col-right-shifted copy: odd[., c] = src[., c-1].
                On ScalarE (ACT) — always 1x there, frees VectorE."""
                o = oddp.tile([128, 2, 260], BF, name=nm, tag=nm[0] + "o")
                nc.scalar.copy(o[:, :, 1:260], src[:, :, 0:259])
                return o

            def rd(ext_tile, k):     # k even
                return ext_tile[:, :, 2 + k : 258 + k]

            def rdo(odd_tile, k):    # k odd
                return odd_tile[:, :, 3 + k : 259 + k]

            def w(name, dtype=BF):
                return scr.tile([128, 2, 260], dtype, name=name, tag="w")

            def stencil(ps, k, rhs3d, start, stop):
                nc.tensor.matmul(ps[:, :, :], stmat[:, k, 0, :], rhs3d,
                                 start=start, stop=False)
                nc.tensor.matmul(ps[:, 0, :], stmat[:, k, 1, :], rhs3d[:, 1, :],
                                 start=False, stop=False)
                nc.tensor.matmul(ps[:, 1, :], stmat[:, k, 1, :], rhs3d[:, 0, :],
                                 start=False, stop=stop)

            STT = nc.vector.scalar_tensor_tensor
            TT = nc.vector.tensor_tensor

            for t in range(T_INT):
                u, v = ut[t], vt[t]
                uo = mkodd(u, f"uo{t}")
                # ---------------- shared per-frame fields
                uc2 = w("uc2")
                uc2v = uc2[:, :, 0:256]
                TT(uc2v, rd(u, 0), rdo(uo, 1), AL.add)
                ucp = w("ucp")[:, :, 0:256]
                nc.vector.tensor_scalar_max(ucp, uc2v, 0.0)
                ucm = w("ucm")[:, :, 0:256]
                nc.vector.tensor_scalar_min(ucm, uc2v, 0.0)

                dvy = psp.tile([128, 2, 256], F32, name="dvy", tag="dvy")
                stencil(dvy, 0, v[:, :, :], True, True)

                dvx = w("dvx")[:, :, 0:256]
                nc.gpsimd.tensor_tensor(dvx, rdo(uo, 1), rdo(uo, -1), AL.subtract)
                dvs4 = w("dvs4")[:, :, 0:256]
                STT(dvs4, dvx, 2.0, dvy, AL.mult, AL.add)

                vcnp = psp.tile([128, 2, 256], F32, name="vcnp", tag="vcn")
                stencil(vcnp, 1, v[:, :, :], True, True)
                vcn2 = w("vcn2")[:, :, 0:256]
                nc.scalar.copy(vcn2, vcnp)

                sq0 = w("sq0")[:, :, 0:256]
                nc.scalar.activation(sq0, dvs4, AF.Square,
                                     accum_out=accs[:, 0, t : t + 1])

                # ---------------- per-field residual
                for fi, ftiles in ((1, ct), (2, dt_)):
                    f0, fprev, fnext = ftiles[t + 1], ftiles[t], ftiles[t + 2]
                    fo = mkodd(f0, f"fo{fi}{t}")
                    fp = w("fp")[:, :, 0:256]
                    STT(fp, rd(f0, 0), 3.0, rdo(fo, -1), AL.mult, AL.subtract)
                    fm = w("fm")[:, :, 0:256]
                    STT(fm, rdo(fo, 1), 3.0, rd(f0, 2), AL.mult, AL.subtract)
                    p = w("p")[:, :, 0:256]
                    TT(p, ucp, fp, AL.mult)
                    q = w("q")[:, :, 0:256]
                    TT(q, ucm, fm, AL.mult)
                    s1 = w("s1")[:, :, 0:256]
                    TT(s1, p, q, AL.add)
                    dd = w("dd")[:, :, 0:256]
                    TT(dd, rdo(fo, 1), rd(f0, 0), AL.subtract)
                    # phiN = 4*beta*D - (p+q) = -phi'
                    phi = w("phi")
                    STT(phi[:, :, 2:258], dd, 4.0 * BETA, s1, AL.mult, AL.subtract)
                    po = oddp.tile([128, 2, 260], BF, name=f"po{fi}{t}", tag="po")
                    nc.scalar.copy(po[:, :, 3:259], phi[:, :, 2:258])
                    nc.scalar.copy(po[:, :, 2:3], phi[:, :, 257:258])
                    # cdx = phi' - phi'_im1 = phiN_im1 - phiN
                    cdx = w("cdx")[:, :, 0:256]
                    TT(cdx, rdo(po, -1), phi[:, :, 2:258], AL.subtract)

                    gp = psp.tile([128, 2, 256], F32, name="gp", tag="gp", bufs=2)
                    stencil(gp, 2, rd(f0, 0), True, True)
                    gm = psp.tile([128, 2, 256], F32, name="gm", tag="gm", bufs=2)
                    stencil(gm, 3, rd(f0, 0), True, True)

                    p2 = w("p2")[:, :, 0:256]
                    STT(p2, vcn2, 0.0, gp, AL.max, AL.mult)
                    q2 = w("q2")[:, :, 0:256]
                    STT(q2, vcn2, 0.0, gm, AL.min, AL.mult)
                    psi = w("psi")[:, :, 0:256]
                    TT(psi, p2, q2, AL.add)

                    py = psp.tile([128, 2, 256], F32, name="py", tag="py", bufs=2)
                    stencil(py, 5, psi, True, False)
                    stencil(py, 4, rd(f0, 0), False, True)

                    t2 = w("t2")[:, :, 0:256]
                    TT(t2, psi, py, AL.subtract)
                    t3 = w("t3")[:, :, 0:256]
                    TT(t3, cdx, t2, AL.add)
                    pd = w("pd")[:, :, 0:256]
                    TT(pd, rd(f0, 0), dvs4, AL.mult)
                    ra = w("ra")[:, :, 0:256]
                    TT(ra, t3, pd, AL.subtract)
                    dtt = w("dtt")[:, :, 0:256]
                    nc.gpsimd.tensor_tensor(dtt, rd(fnext, 0), rd(fprev, 0),
                                            AL.subtract)
                    rr = w("rr")[:, :, 0:256]
                    STT(rr, dtt, KT, ra, AL.mult, AL.add)
                    sqf = w("sqf")[:, :, 0:256]
                    nc.scalar.activation(sqf, rr, AF.Square,
                                         accum_out=accs[:, fi, t : t + 1])

            # ---------------- final reduction to [1, 3]
            red3 = constp.tile([128, 3], F32, name="red3")
            for k in range(3):
                nc.vector.tensor_reduce(red3[:, k : k + 1], accs[:, k, :],
                                        mybir.AxisListType.X, AL.add)
            psr = psp.tile([1, 4], F32, name="psr", tag="dvy")
            nc.tensor.matmul(psr[:, 0:3], ones, red3, start=True, stop=True)
            outt = constp.tile([1, 4], F32, name="outt")
            nc.vector.memset(outt, 0.0)
            nc.scalar.copy(outt[:, 0:3], psr[:, 0:3])
            nc.sync.dma_start(out=out_ext[:, :], in_=outt)

    nc.compile()
    return nc


def _get_nc():
    if "nc" not in _CACHE:
        _CACHE["nc"] = _build()
        _CACHE["stmat"] = _stencil_mats()
    return _CACHE["nc"]


def _make_in_maps(u, v, c, Dxx):
    u = np.ascontiguousarray(np.asarray(u, dtype=np.float32))
    v = np.ascontiguousarray(np.asarray(v, dtype=np.float32))
    c = np.ascontiguousarray(np.asarray(c, dtype=np.float32))
    Dxx = np.ascontiguousarray(np.asarray(Dxx, dtype=np.float32))
    stmat = _CACHE["stmat"]
    in_maps = []
    for i in range(N_CORES):
        b, h = i // 2, i % 2
        t0 = 1 + T_INT * h
        in_maps.append({
            "u": np.ascontiguousarray(u[b, t0 : t0 + T_INT]),
            "v": np.ascontiguousarray(v[b, t0 : t0 + T_INT]),
            "c": np.ascontiguousarray(c[b, t0 - 1 : t0 + T_INT + 1]),
            "d": np.ascontiguousarray(Dxx[b, t0 - 1 : t0 + T_INT + 1]),
            "stmat": stmat,
        })
    return in_maps


def _combine(results):
    s = np.zeros(3, dtype=np.float64)
    for r in results:
        s += np.asarray(r["out"], dtype=np.float64)[0, :3]
    n = B * (T - 2) * H * W
    loss_cont = s[0] / (16.0 * DX * DX * n)
    loss_conc = s[1] / (16.0 * DX * DX * n)
    loss_dxx = s[2] / (16.0 * DX * DX * n)
    total = loss_cont + loss_conc + loss_dxx
    return np.array([total, loss_cont, loss_conc, loss_dxx], dtype=np.float32)


def kernel(u, v, c, Dxx):
    nc = _get_nc()
    in_maps = _make_in_maps(u, v, c, Dxx)
    res = run_bass_kernel_spmd(nc, in_maps, core_ids=list(range(N_CORES)))
    return _combine(res.results)


if __name__ == "__main__":
    rng = np.random.default_rng(0)
    inputs = {
        "u": rng.standard_normal((B, T, H, W), dtype=np.float32),
        "v": rng.standard_normal((B, T, H, W), dtype=np.float32),
        "c": rng.random((B, T, H, W), dtype=np.float32),
        "Dxx": rng.random((B, T, H, W), dtype=np.float32),
    }
    print(kernel(**inputs))
